# revision 54
# baseline (speedup 1.0000x reference)
"""Trainium2 Bass kernel for the DGCNN (gnn_message_passing) problem.

Strategy: data-parallel over the graph batch — 125 graphs per NeuronCore
(8 cores). Host does only layout/integer work: shards/transposes inputs,
converts the COO edge list into per-graph dense adjacency count matrices and
their integer row-sums (degrees), gathers the z-embedding rows (pure layout),
and packs the small weights into two SBUF-resident blobs (one fp32, one
fp32r) loaded with a single DMA each. All model FLOAT math (degree
normalization, GCN layers, sort-pooling, convolutions, MLP) runs on device:
fp32 for everything feeding the sort keys (sort-pool ordering is numerically
fragile), fp32r only for post-sort convolutions.

Per-core schedule: 7 blocks of 16 graphs + tail blocks of 8 and 5 graphs
(small tail blocks shorten the drain), software-pipelined two stages deep:
block b's GCN matmuls (PE) overlap block b-1's top-K index roundtrip +
feature gathers (SP-DMA/Pool) and block b-2's conv1 (PE) — conv1 is
deferred two blocks so its gathered inputs are always long ready and PE
never stalls on the gather chain. Input loads are issued mid-GCN on the
Pool SWDGE path (zT/xblk) and pre-laid-out on host so each is few-
descriptor; weights arrive as three blob DMAs. PE queue order/iteration:
  gcn(b) -> xbuild transposes(b+1) -> conv1(b-2) -> tail part
Per block:
  X = D A^T D (DVE/ACT scale, PE transpose, scaled evac) -> 3 GCN layers
  (per-graph fp32 matmuls, 4-graph PSUM groups) -> layer-3 sort key -> PE
  transpose -> top-40 via DVE max/max_index/match_replace (5 rounds of
  top-8, exact fp32, reference tie order) -> int16 index roundtrip through
  DRAM into ap_gather layout -> feature gathers + fp32r casts (GPSIMD) ->
  fp32r conv1 (320-col chunks).
Tail (maxpool -> fp32r conv2 -> partial MLP + per-part finish incl the
final y DMA) runs in per-graph-independent parts so the last 32-graph slab
splits around the final block's gather chain.
"""
import numpy as np
import concourse.bass as bass
import concourse.mybir as mybir
import concourse.tile as tile
from concourse.masks import make_identity

F32 = mybir.dt.float32
F32R = mybir.dt.float32r
BF16 = mybir.dt.bfloat16
I16 = mybir.dt.int16
U32 = mybir.dt.uint32
AF = mybir.ActivationFunctionType
ALU = mybir.AluOpType

NP_ = 100
K = 40
GS = 128  # graph slots in tail
BG = 16   # graphs per block
NH = BG * K  # 640 top-K slots per block

# fp32 weight blob column layout
BL_W0A = 0
BL_W0B = 128
BL_W1 = 256
BL_W2 = 384
BL_W3 = 512
BL_B0 = 513
BL_B1 = 514
BL_B2 = 515
BL_MB1 = 516
BL_MW2 = 517
BL_GBASE = 518
BL_CB1 = 519
BL_CB2 = 520
BF_COLS = 521         # main fp32 blob (everything except mW1)
BM_COLS = 16 * 128    # mW1 blob: [0:32, 128*t : 128*(t+1)]

# fp32r blob layout
BR_CW1C = 0           # [128, 0:48]
BR_CW1C3 = 48         # [0:1, 48:64]
BR_CW2 = 64           # [0:16, 64+32j : 96+32j]
BR_COLS = 64 + 5 * 32


def ceil16(n):
    return (n + 15) // 16 * 16


# ======================================================================
# host packing
# ======================================================================
def pack_core_inputs(x, z, counts_ds, weights):
    """x [N,128] f32, z [N] int, counts_ds [G,100,100] (counts[g,d,s]),
    weights: dict of the model params (numpy). Returns in_map for the kernel."""
    import ml_dtypes
    N = x.shape[0]
    m = {}
    m["xT"] = np.ascontiguousarray(x.T.astype(np.float32))
    # z-embedding rows (pure gather; float values copied bit-exactly)
    m["zeT"] = np.ascontiguousarray(
        weights["z_table"].astype(np.float32)[np.asarray(z).astype(np.int64)].T)
    # adjacency pre-laid-out in the on-chip at4 tile format:
    # Atw[d, g*100+s] = counts[g, d, s] -> per-block DMA is 100 contiguous
    # descriptors instead of 1600 (SWDGE-ring friendly)
    m["Atw"] = np.ascontiguousarray(
        counts_ds.transpose(1, 0, 2).reshape(NP_, N).astype(ml_dtypes.bfloat16))
    deg = counts_ds.sum(axis=2).astype(np.float32)  # [G, 100]
    m["degT"] = np.ascontiguousarray(deg.T)         # [100, G]

    bf = np.zeros((128, BF_COLS), np.float32)
    W0 = weights["W0"].astype(np.float32)
    bf[:, BL_W0A:BL_W0A + 128] = W0[:128]
    bf[:, BL_W0B:BL_W0B + 128] = W0[128:]
    bf[:, BL_W1:BL_W1 + 128] = weights["W1"].astype(np.float32)
    bf[:, BL_W2:BL_W2 + 128] = weights["W2"].astype(np.float32)
    bf[:, BL_W3] = weights["W3"].astype(np.float32).reshape(128)
    bf[:, BL_B0] = weights["b0"].astype(np.float32)
    bf[:, BL_B1] = weights["b1"].astype(np.float32)
    bf[:, BL_B2] = weights["b2"].astype(np.float32)
    bf[:, BL_MB1] = weights["mb1"].astype(np.float32)
    bf[:, BL_MW2] = weights["mW2"].astype(np.float32).reshape(128)
    bf[:16, BL_GBASE] = np.arange(16) * NP_
    bf[:16, BL_CB1] = weights["cb1"].astype(np.float32)
    bf[:32, BL_CB2] = weights["cb2"].astype(np.float32)
    m["blobf"] = bf
    bm = np.zeros((32, BM_COLS), np.float32)
    mW1 = weights["mW1"].astype(np.float32)  # [512, 128], row index = o*16+t
    for t in range(16):
        bm[:, 128 * t: 128 * t + 128] = mW1[np.arange(32) * 16 + t]
    m["blobm"] = bm

    br = np.zeros((128, BR_COLS), np.float32)
    cw1 = weights["cw1"].astype(np.float32)[:, 0, :]  # [16, 385]
    for c in range(3):
        br[:, BR_CW1C + 16 * c: BR_CW1C + 16 * c + 16] = \
            cw1[:, 128 * c:128 * (c + 1)].T
    br[0, BR_CW1C3:BR_CW1C3 + 16] = cw1[:, 384]
    cw2 = weights["cw2"].astype(np.float32)  # [32, 16, 5]
    for j in range(5):
        br[:16, BR_CW2 + 32 * j: BR_CW2 + 32 * j + 32] = cw2[:, :, j].T
    m["blobr"] = br
    return m, float(weights["b3"][0]), float(weights["mb2"][0])


# ======================================================================
# kernel builder
# ======================================================================
class _Ctx:
    pass


def build_kernel(nc, G, b3_val, mb2_val):
    N = G * NP_
    dt = {}
    def din(name, shape, dtype=F32):
        dt[name] = nc.dram_tensor(name, shape, dtype, kind="ExternalInput")
        return dt[name]

    din("xT", [128, N])
    din("zeT", [128, N])
    din("Atw", [NP_, N], BF16)
    din("degT", [NP_, G])
    din("blobf", [128, BF_COLS])
    din("blobm", [32, BM_COLS])
    din("blobr", [128, BR_COLS])
    yout = nc.dram_tensor("y", [1, GS], F32, kind="ExternalOutput")

    with tile.TileContext(nc) as tc:
        with tc.tile_pool(name="wp", bufs=1) as wp, \
             tc.tile_pool(name="persist", bufs=1) as pp, \
             tc.tile_pool(name="dscr", bufs=2, space="DRAM") as dp:
            # degT first (prologue dinv chain is on the early critical path)
            degA = pp.tile([NP_, 128], F32, tag="degA")
            nc.sync.dma_start(out=degA[:, :G], in_=dt["degT"][:])
            blobf = wp.tile([128, BF_COLS], F32)
            blobm = wp.tile([32, BM_COLS], F32)
            blobr = wp.tile([128, BR_COLS], F32R)

            W = {}
            W["W0a"] = blobf[:, BL_W0A:BL_W0A + 128]
            W["W0b"] = blobf[:, BL_W0B:BL_W0B + 128]
            W["W1"] = blobf[:, BL_W1:BL_W1 + 128]
            W["W2"] = blobf[:, BL_W2:BL_W2 + 128]
            W["w3"] = blobf[:, BL_W3:BL_W3 + 1]
            W["b0"] = blobf[:, BL_B0:BL_B0 + 1]
            W["b1"] = blobf[:, BL_B1:BL_B1 + 1]
            W["b2"] = blobf[:, BL_B2:BL_B2 + 1]
            W["mb1"] = blobf[:, BL_MB1:BL_MB1 + 1]
            W["mW2"] = blobf[:, BL_MW2:BL_MW2 + 1]
            W["gbase"] = blobf[:16, BL_GBASE:BL_GBASE + 1]
            W["cb1"] = blobf[:16, BL_CB1:BL_CB1 + 1]
            W["cb2"] = blobf[:32, BL_CB2:BL_CB2 + 1]
            for t_ in range(16):
                W[f"mW1s_{t_}"] = blobm[:, 128 * t_: 128 * t_ + 128]
            W["cw1c"] = blobr[:, BR_CW1C:BR_CW1C + 48]
            W["cw1c3"] = blobr[:1, BR_CW1C3:BR_CW1C3 + 16]
            for j in range(5):
                W[f"cw2j_{j}"] = blobr[:16, BR_CW2 + 32 * j:BR_CW2 + 32 * j + 32]

            ident = wp.tile([128, 128], F32)
            make_identity(nc, ident[:])

            # ---- prologue: dinv for all graphs (one Sqrt table switch) ----
            dinvA = pp.tile([NP_, 128], F32, tag="dinvA")
            dmask = pp.tile([NP_, 128], F32, tag="dmask")
            nc.vector.tensor_scalar(dmask[:, :G], degA[:, :G], 0.5, None, op0=ALU.is_ge)
            nc.vector.tensor_scalar_max(dinvA[:, :G], degA[:, :G], 1.0)
            nc.vector.reciprocal(dinvA[:, :G], dinvA[:, :G])
            nc.scalar.activation(dinvA[:, :G], dinvA[:, :G], AF.Sqrt)
            nc.vector.tensor_mul(dinvA[:, :G], dinvA[:, :G], dmask[:, :G])

            # Y1 persistent [16, GS*K]; only pad graph slots need zeroing
            Y1 = pp.tile([16, GS * K], F32, tag="Y1")
            if G * K < GS * K:
                nc.vector.memset(Y1[:, G * K:], 0.0)

            with tc.tile_pool(name="blk", bufs=2) as blk, \
                 tc.tile_pool(name="ld", bufs=4) as ld, \
                 tc.tile_pool(name="sm", bufs=2) as sm, \
                 tc.tile_pool(name="grp", bufs=3) as grp, \
                 tc.tile_pool(name="gth", bufs=3) as gth, \
                 tc.tile_pool(name="gta", bufs=1) as gta, \
                 tc.tile_pool(name="st", bufs=2) as st, \
                 tc.tile_pool(name="pt", bufs=2, space="PSUM") as ptp, \
                 tc.tile_pool(name="phw", bufs=2, space="PSUM") as phw, \
                 tc.tile_pool(name="pag", bufs=2, space="PSUM") as pag, \
                 tc.tile_pool(name="pms", bufs=1, space="PSUM") as pms:
                cx = _Ctx()
                cx.nc, cx.tc, cx.dt, cx.W, cx.ident, cx.Y1, cx.dp = \
                    nc, tc, dt, W, ident, Y1, dp
                cx.blk, cx.sm, cx.grp, cx.gth, cx.st = blk, sm, grp, gth, st
                cx.ld = ld
                cx.gta = gta
                cx.ptp, cx.phw, cx.pag, cx.pms = ptp, phw, pag, pms
                cx.dinvA = dinvA
                cx.b3, cx.mb2 = float(b3_val), float(mb2_val)
                cx.zps = pms.tile([128, 128], F32, tag="zps")
                cx.yout = yout

                # block sizes: full 16-graph blocks, remainder split into two
                # small sub-blocks so the final drain chain covers few graphs
                bcnts = [BG] * (G // BG)
                rem = G - BG * (G // BG)
                if rem > 8:
                    bcnts += [8, rem - 8]
                elif rem > 0:
                    bcnts += [rem]
                nblk = len(bcnts)
                g0s = [sum(bcnts[:b]) for b in range(nblk)]
                cx.state = [dict() for _ in range(nblk)]
                nchunks = (G + 31) // 32

                # block-0 loads on the SP/HWDGE path, interleaved with the
                # weight blobs in first-use order (the DMA engines serialize
                # transfers, so issue order = arrival order): at4 gates the
                # X build, zT + W0a gate the first L0 matmul, xblk its
                # accumulate pass; blobr/blobm are needed much later.
                ncols0 = bcnts[0] * NP_
                at40 = cx.ld.tile([NP_, BG * NP_], BF16, tag="at4")
                nc.sync.dma_start(out=at40[:, :ncols0], in_=dt["Atw"][:, :ncols0])
                zT0 = cx.ld.tile([128, BG * NP_], F32, tag="zT")
                nc.sync.dma_start(out=zT0[:, :ncols0], in_=dt["zeT"][:, :ncols0])
                nc.sync.dma_start(out=blobf[:], in_=dt["blobf"][:])
                xblk0 = cx.ld.tile([128, BG * NP_], F32, tag="xblk")
                nc.sync.dma_start(out=xblk0[:, :ncols0], in_=dt["xT"][:, :ncols0])
                nc.sync.dma_start(out=blobr[:], in_=dt["blobr"][:].bitcast(F32R))
                nc.sync.dma_start(out=blobm[:], in_=dt["blobm"][:])
                cx.state[0]["at4"] = at40
                cx.state[0]["zT"] = zT0
                cx.state[0]["xblk"] = xblk0
                _emit_xbuild(cx, 0, g0s[0], bcnts[0])
                done_topk = 0
                next_chunk = 0
                for it in range(nblk):
                    if it == 0 and 1 < nblk:
                        _emit_loads(cx, 1, g0s[1], bcnts[1])
                    # block it+1's loads are emitted mid-GCN so their (DMA-
                    # engine-serialized) transfers land between the per-block
                    # index-roundtrip windows
                    if it >= 1 and it + 1 < nblk:
                        mid = (lambda j: (lambda: _emit_loads(
                            cx, j, g0s[j], bcnts[j])))(it + 1)
                    else:
                        mid = None
                    _emit_gcn(cx, it, bcnts[it], last=(it == nblk - 1), mid=mid)
                    if it + 1 < nblk:
                        _emit_xbuild(cx, it + 1, g0s[it + 1], bcnts[it + 1])
                    if it >= 2:
                        _emit_conv1(cx, it - 2, g0s[it - 2], bcnts[it - 2])
                        done_topk += bcnts[it - 2]
                        while (next_chunk < nchunks
                               and done_topk >= min(32 * (next_chunk + 1), G)):
                            _emit_tail_chunk(cx, next_chunk)
                            next_chunk += 1
                    if it >= 1:
                        _emit_gath(cx, it - 1, bcnts[it - 1],
                                   last=(it - 1 == nblk - 2))
                # final: gathers for the last block start (they wait on its
                # index roundtrip); meanwhile PE does block nblk-2's conv1 and
                # the ready part of the last tail slab, then the tiny rest.
                _emit_gath(cx, nblk - 1, bcnts[nblk - 1], last=True)
                _emit_conv1(cx, nblk - 2, g0s[nblk - 2], bcnts[nblk - 2])
                done_topk += bcnts[nblk - 2]
                while (next_chunk < nchunks
                       and done_topk >= min(32 * (next_chunk + 1), G)):
                    _emit_tail_chunk(cx, next_chunk)
                    next_chunk += 1
                cA = 32 * next_chunk
                partA = done_topk - cA
                if next_chunk < nchunks and partA > 0:
                    _emit_tail_chunk(cx, next_chunk, 0, partA)
                _emit_conv1(cx, nblk - 1, g0s[nblk - 1], bcnts[nblk - 1])
                done_topk += bcnts[nblk - 1]
                if next_chunk < nchunks:
                    rest = min(32, GS - cA) - partA
                    _emit_tail_chunk(cx, next_chunk, partA, rest)
                    next_chunk += 1
    return yout


def _emit_loads(cx, b, g0, bcnt, eng=None):
    """Issue block b's big DMAs on the Pool SWDGE path (bypasses HWDGE)."""
    nc = cx.nc
    eng = eng or nc.gpsimd
    n0 = g0 * NP_
    ncols = bcnt * NP_
    at4 = cx.ld.tile([NP_, BG * NP_], BF16, tag="at4")
    eng.dma_start(out=at4[:, :ncols], in_=cx.dt["Atw"][:, n0:n0 + ncols])
    zT = cx.ld.tile([128, BG * NP_], F32, tag="zT")
    eng.dma_start(out=zT[:, :ncols], in_=cx.dt["zeT"][:, n0:n0 + ncols])
    xblk = cx.ld.tile([128, BG * NP_], F32, tag="xblk")
    eng.dma_start(out=xblk[:, :ncols], in_=cx.dt["xT"][:, n0:n0 + ncols])
    cx.state[b]["at4"] = at4
    cx.state[b]["zT"] = zT
    cx.state[b]["xblk"] = xblk


def _emit_xbuild(cx, b, g0, bcnt):
    """X = D A^T D for block b."""
    nc = cx.nc
    s = cx.state[b]
    at4 = s["at4"]

    Xall = cx.blk.tile([NP_, BG * NP_], F32, tag="Xall")
    for g in range(bcnt):
        dcol = cx.dinvA[:, g0 + g:g0 + g + 1]
        bds = cx.grp.tile([NP_, NP_], F32, tag="bds")
        if g % 2 == 0:
            nc.vector.tensor_scalar_mul(bds[:], at4[:, g * NP_:(g + 1) * NP_], dcol)
        else:
            nc.scalar.activation(bds[:], at4[:, g * NP_:(g + 1) * NP_],
                                 AF.Copy, scale=dcol)
        pt = cx.ptp.tile([128, 128], F32, tag="ptrans")
        nc.tensor.transpose(pt[:NP_, :NP_], bds[:], cx.ident[:NP_, :NP_])
        if g % 2 == 0:
            nc.scalar.activation(Xall[:, g * NP_:(g + 1) * NP_], pt[:NP_, :NP_],
                                 AF.Copy, scale=dcol)
        else:
            nc.vector.tensor_scalar_mul(Xall[:, g * NP_:(g + 1) * NP_],
                                        pt[:NP_, :NP_], dcol)
    s["Xall"] = Xall


def _emit_gcn(cx, b, bcnt, last=False, mid=None):
    """GCN layers + sort keys + top-40 + index roundtrip for block b."""
    nc, W = cx.nc, cx.W
    s = cx.state[b]
    Xall, zT, xblk = s["Xall"], s["zT"], s["xblk"]

    h1 = cx.blk.tile([128, BG * NP_], F32, tag="h1")
    h2 = cx.blk.tile([128, BG * NP_], F32, tag="h2")
    h3 = cx.blk.tile([128, BG * NP_], F32, tag="h3")
    hs = [h1, h2, h3]
    s["hs"] = hs
    for layer in range(3):
        Wl = [None, W["W1"], W["W2"]][layer]
        bl = [W["b0"], W["b1"], W["b2"]][layer]
        if layer == 1 and mid is not None:
            mid()
        for g4 in range(0, bcnt, 4):
            gcnt = min(4, bcnt - g4)
            hwp = cx.phw.tile([128, 512], F32, tag="hw4")
            for i in range(gcnt):
                g = g4 + i
                sl = slice(g * NP_, (g + 1) * NP_)
                osl = slice(i * 128, i * 128 + 128)
                if layer == 0:
                    nc.tensor.matmul(hwp[:NP_, osl], lhsT=zT[:, sl],
                                     rhs=W["W0a"], start=True, stop=False)
                    nc.tensor.matmul(hwp[:NP_, osl], lhsT=xblk[:, sl],
                                     rhs=W["W0b"], start=False, stop=True)
                else:
                    nc.tensor.matmul(hwp[:NP_, osl], lhsT=hs[layer - 1][:, sl],
                                     rhs=Wl, start=True, stop=True)
            P4 = cx.grp.tile([128, 512], F32, tag="P4")
            if layer == 1:
                nc.scalar.activation(P4[:NP_, :gcnt * 128], hwp[:NP_, :gcnt * 128],
                                     AF.Copy)
            else:
                nc.vector.tensor_copy(P4[:NP_, :gcnt * 128], hwp[:NP_, :gcnt * 128])
            agg = cx.pag.tile([128, 512], F32, tag="agg")
            for i in range(gcnt):
                g = g4 + i
                nc.tensor.matmul(agg[:, i * NP_:(i + 1) * NP_],
                                 lhsT=P4[:NP_, i * 128:(i + 1) * 128],
                                 rhs=Xall[:, g * NP_:(g + 1) * NP_],
                                 start=True, stop=True)
            nc.scalar.activation(hs[layer][:, g4 * NP_: (g4 + gcnt) * NP_],
                                 agg[:, :gcnt * NP_], AF.Tanh, bias=bl)

    # ---- layer 3: per-node sort key (pre-tanh) ----
    vps = cx.phw.tile([128, 512], F32, tag="hw4")
    for i in range(bcnt):
        nc.tensor.matmul(vps[:NP_, i:i + 1], lhsT=h3[:, i * NP_:(i + 1) * NP_],
                         rhs=W["w3"], start=True, stop=True)
    vsb = cx.sm.tile([NP_, BG], F32, tag="vsb")
    nc.vector.tensor_copy(vsb[:, :bcnt], vps[:NP_, :bcnt])
    h4ps = cx.pag.tile([128, 512], F32, tag="agg")
    for i in range(bcnt):
        nc.tensor.matmul(h4ps[:NP_, i:i + 1], lhsT=Xall[:, i * NP_:(i + 1) * NP_],
                         rhs=vsb[:, i:i + 1], start=True, stop=True)
    h4blk = cx.sm.tile([NP_, BG], F32, tag="h4blk")
    nc.scalar.activation(h4blk[:, :bcnt], h4ps[:NP_, :bcnt], AF.Copy, bias=cx.b3)

    # ---- transpose keys to [graphs, nodes], top-40 via 5x top-8 ----
    h4Tps = cx.ptp.tile([128, 128], F32, tag="ptrans")
    nc.tensor.transpose(h4Tps[:bcnt, :NP_], h4blk[:, :bcnt], cx.ident[:NP_, :NP_])
    h4T = cx.sm.tile([BG, NP_], F32, tag="h4T")
    nc.vector.tensor_copy(h4T[:bcnt, :], h4Tps[:bcnt, :NP_])

    vals = cx.sm.tile([BG, K], F32, tag="vals")
    idxs32 = cx.sm.tile([BG, K], U32, tag="idxs32")
    idx16 = cx.sm.tile([BG, K], I16, tag="idx16")
    idxf = cx.sm.tile([BG, K], F32, tag="idxf")
    scr_idx = cx.dp.tile([BG, K], I16, tag="scr_idx")
    if bcnt < BG:
        nc.vector.memset(vals[:], 0.0)
        nc.vector.memset(idx16[:], 0)
    for r in range(5):
        nc.vector.max(vals[:bcnt, 8 * r:8 * r + 8], h4T[:bcnt, :])
        nc.vector.max_index(idxs32[:bcnt, 8 * r:8 * r + 8],
                            vals[:bcnt, 8 * r:8 * r + 8], h4T[:bcnt, :])
        if r < 4:
            nc.vector.match_replace(h4T[:bcnt, :], vals[:bcnt, 8 * r:8 * r + 8],
                                    h4T[:bcnt, :], -1e30)
        if last:
            # final block: convert + write this round's 8 index columns
            # immediately so the scratch-write latency hides under the rounds
            sl8 = slice(8 * r, 8 * r + 8)
            nc.vector.tensor_copy(idxf[:bcnt, sl8], idxs32[:bcnt, sl8])
            nc.vector.tensor_scalar_add(idxf[:bcnt, sl8], idxf[:bcnt, sl8],
                                        W["gbase"][:bcnt, :])
            nc.vector.tensor_copy(idx16[:bcnt, sl8], idxf[:bcnt, sl8])
            nc.sync.dma_start(out=scr_idx[:, sl8], in_=idx16[:, sl8])

    if not last:
        nc.vector.tensor_copy(idxf[:bcnt, :], idxs32[:bcnt, :])
        nc.vector.tensor_scalar_add(idxf[:bcnt, :], idxf[:bcnt, :],
                                    W["gbase"][:bcnt, :])
        nc.vector.tensor_copy(idx16[:bcnt, :], idxf[:bcnt, :])

    # ---- roundtrip through DRAM: wrapped idx layout + flat vals ----
    if not last:
        nc.sync.dma_start(out=scr_idx[:], in_=idx16[:])
    iw128 = cx.sm.tile([128, NH // 16], I16, tag="iw128")
    wrapped = scr_idx[:].rearrange("g k -> (g k)").rearrange("(c p) -> p c", p=16)
    for c in range(8):
        eng = nc.gpsimd if (last and c >= 5) else nc.sync
        eng.dma_start(out=iw128[16 * c:16 * (c + 1), :], in_=wrapped)

    nc.scalar.activation(vals[:bcnt, :], vals[:bcnt, :], AF.Tanh)
    scr_val = cx.dp.tile([BG, K], F32, tag="scr_val")
    nc.sync.dma_start(out=scr_val[:], in_=vals[:])
    h4row = cx.ld.tile([1, NH], F32R, tag="h4row")
    nc.sync.dma_start(out=h4row[:],
                      in_=scr_val[:].rearrange("g k -> (g k)").rearrange(
                          "(o n) -> o n", o=1).bitcast(F32R))
    s["iw128"], s["h4row"] = iw128, h4row


def _emit_gath(cx, b, bcnt, last=False):
    """Feature gathers for block b (Pool; consumed by conv1 a block later)."""
    nc = cx.nc
    s = cx.state[b]
    ncols = bcnt * NP_
    nki = ceil16(bcnt * K)
    grs = []
    for c in range(3):
        gat = cx.gta.tile([128, NH], F32, tag=f"gath{c}")
        nc.gpsimd.ap_gather(gat[:, :nki], s["hs"][c][:, :ncols],
                            s["iw128"][:, :nki // 16],
                            channels=128, num_elems=ncols, d=1, num_idxs=nki)
        gr = cx.gth.tile([128, NH], F32R, tag=f"gr{c}")
        # final block: DVE is idle at the drain, so the f32r casts pipeline
        # against the next gather instead of serializing on Pool
        ceng = nc.vector if last else nc.gpsimd
        ceng.tensor_copy(gr[:, :nki], gat[:, :nki])
        grs.append(gr)
    s["grs"] = grs


def _emit_conv1(cx, b, g0, bcnt):
    """conv1 for block b (gathers ran a block earlier -> PE never stalls)."""
    nc, W = cx.nc, cx.W
    s = cx.state[b]
    grs = s["grs"]
    nk = bcnt * K
    hof = g0 * K
    for ch0 in range(0, nk, 320):
        cw = min(320, nk - ch0)
        cps = cx.phw.tile([128, 512], F32, tag="hw4")
        for c in range(3):
            nc.tensor.matmul(cps[:16, :cw], lhsT=W["cw1c"][:, 16 * c:16 * (c + 1)],
                             rhs=grs[c][:, ch0:ch0 + cw], start=(c == 0), stop=False)
        nc.tensor.matmul(cps[:16, :cw], lhsT=W["cw1c3"],
                         rhs=s["h4row"][:1, ch0:ch0 + cw], start=False, stop=True)
        nc.scalar.activation(cx.Y1[:, hof + ch0: hof + ch0 + cw],
                             cps[:16, :cw], AF.Relu, bias=W["cb1"])


def _emit_tail_chunk(cx, ch, goff=0, gcnt=32):
    """Maxpool + conv2 + partial mlp1 + finish for graph slots
    [32*ch+goff, 32*ch+goff+gcnt) (per-graph independent -> splittable)."""
    nc, W = cx.nc, cx.W
    g0 = 32 * ch + goff
    Y2c = cx.st.tile([16, 640], F32R, tag="Y2c")
    y1v = cx.Y1[:, 40 * g0:40 * (g0 + gcnt)].rearrange(
        "p (m two) -> p m two", two=2)
    nc.vector.tensor_tensor(Y2c[:, :20 * gcnt], y1v[:, :, 0], y1v[:, :, 1],
                            op=ALU.max)

    # conv2: 5 shifted matmuls over the part's graph slots; pad tiny parts
    # to 16 slots so the fp32r matmul hits the >=256-free-dim fast path
    # (padded slots read stale-but-finite Y2c cols and are never consumed)
    gmm = max(gcnt, 16)
    cps = cx.pms.tile([32, 512], F32, tag="cps")
    for j in range(5):
        rhs = Y2c[:, :20 * gmm].rearrange("p (g t) -> p g t", t=20)[:, :, j:j + 16]
        nc.tensor.matmul(cps[:, :16 * gmm], lhsT=W[f"cw2j_{j}"], rhs=rhs,
                         start=(j == 0), stop=(j == 4))
    Y3c = cx.st.tile([32, 512], F32, tag="Y3c")
    nc.scalar.activation(Y3c[:, :16 * gcnt], cps[:, :16 * gcnt],
                         AF.Relu, bias=W["cb2"])

    # mlp1 partial: accumulate this part's columns of zps
    for t in range(16):
        rhs = Y3c[:, :16 * gcnt].rearrange("p (g t) -> p g t", t=16)[:, :, t]
        nc.tensor.matmul(cx.zps[:, g0:g0 + gcnt],
                         lhsT=W[f"mW1s_{t}"], rhs=rhs,
                         start=(t == 0), stop=(t == 15))

    # finish this part's graphs: relu -> mW2 -> +mb2 -> DMA out
    Z1 = cx.st.tile([128, 32], F32, tag="Z1")
    nc.scalar.activation(Z1[:, :gcnt], cx.zps[:, g0:g0 + gcnt],
                         AF.Relu, bias=W["mb1"])
    ypst = cx.pms.tile([32, 512], F32, tag="cps")
    nc.tensor.matmul(ypst[:1, :gcnt], lhsT=W["mW2"], rhs=Z1[:, :gcnt],
                     start=True, stop=True)
    ysb = cx.st.tile([1, 32], F32, tag="ysb")
    nc.scalar.activation(ysb[:, :gcnt], ypst[:1, :gcnt], AF.Copy, bias=cx.mb2)
    nc.sync.dma_start(out=cx.yout[:1, g0:g0 + gcnt], in_=ysb[:, :gcnt])


# ======================================================================
# public entry point: kernel(**inputs) -> np.ndarray [1000, 1]
# ======================================================================
NUM_CORES = 8
NUM_GRAPHS = 1000
G_PER_CORE = NUM_GRAPHS // NUM_CORES  # 125

_cache = {}


def _get_compiled(b3_val, mb2_val):
    key = "nc"
    if key not in _cache:
        from concourse import bacc
        nc = bacc.Bacc("TRN2", target_bir_lowering=False, debug=False,
                       num_devices=NUM_CORES, dynamic_dma_scratch_size=16384)
        build_kernel(nc, G_PER_CORE, b3_val, mb2_val)
        nc.compile()
        _cache[key] = nc
    return _cache[key]


def kernel(x, z, edge_index, batch, z_table, W0, b0, W1, b1, W2, b2, W3, b3,
           cw1, cb1, cw2, cb2, mW1, mb1, mW2, mb2, **_unused):
    x = np.asarray(x, np.float32)
    z = np.asarray(z)
    ei = np.asarray(edge_index)
    weights = dict(z_table=np.asarray(z_table), W0=np.asarray(W0),
                   b0=np.asarray(b0), W1=np.asarray(W1), b1=np.asarray(b1),
                   W2=np.asarray(W2), b2=np.asarray(b2), W3=np.asarray(W3),
                   b3=np.asarray(b3), cw1=np.asarray(cw1), cb1=np.asarray(cb1),
                   cw2=np.asarray(cw2), cb2=np.asarray(cb2), mW1=np.asarray(mW1),
                   mb1=np.asarray(mb1), mW2=np.asarray(mW2), mb2=np.asarray(mb2))

    # dense per-graph adjacency counts: row (g, d), col s  (edges are intra-graph)
    src = ei[0].astype(np.int64)
    dst = ei[1].astype(np.int64)
    code = dst * NP_ + (src % NP_)  # == (g*100 + d)*100 + s
    counts = np.bincount(code, minlength=NUM_GRAPHS * NP_ * NP_).astype(np.float32)
    counts = counts.reshape(NUM_GRAPHS, NP_, NP_)

    b3v = float(np.asarray(b3).reshape(-1)[0])
    mb2v = float(np.asarray(mb2).reshape(-1)[0])
    nc = _get_compiled(b3v, mb2v)

    in_maps = []
    for c in range(NUM_CORES):
        g0 = c * G_PER_CORE
        n0 = g0 * NP_
        n1 = n0 + G_PER_CORE * NP_
        m, _, _ = pack_core_inputs(x[n0:n1], z[n0:n1], counts[g0:g0 + G_PER_CORE],
                                   weights)
        in_maps.append(m)

    from concourse.bass_utils import run_bass_kernel_spmd
    res = run_bass_kernel_spmd(nc, in_maps, list(range(NUM_CORES)))
    y = np.concatenate([res.results[c]["y"][0, :G_PER_CORE]
                        for c in range(NUM_CORES)])
    return y.reshape(NUM_GRAPHS, 1).astype(np.float32)


# revision 56
# speedup vs baseline: 1.0081x; 1.0081x over previous
"""Trainium2 Bass kernel for the DGCNN (gnn_message_passing) problem.

Strategy: data-parallel over the graph batch — 125 graphs per NeuronCore
(8 cores). Host does only layout/integer work: shards/transposes inputs,
converts the COO edge list into per-graph dense adjacency count matrices and
their integer row-sums (degrees), gathers the z-embedding rows (pure layout),
and packs the small weights into two SBUF-resident blobs (one fp32, one
fp32r) loaded with a single DMA each. All model FLOAT math (degree
normalization, GCN layers, sort-pooling, convolutions, MLP) runs on device:
fp32 for everything feeding the sort keys (sort-pool ordering is numerically
fragile), fp32r only for post-sort convolutions.

Per-core schedule: 7 blocks of 16 graphs + tail blocks of 8 and 5 graphs
(small tail blocks shorten the drain), software-pipelined two stages deep:
block b's GCN matmuls (PE) overlap block b-1's top-K index roundtrip +
feature gathers (SP-DMA/Pool) and block b-2's conv1 (PE) — conv1 is
deferred two blocks so its gathered inputs are always long ready and PE
never stalls on the gather chain. Input loads are issued mid-GCN on the
Pool SWDGE path (zT/xblk) and pre-laid-out on host so each is few-
descriptor; weights arrive as three blob DMAs. PE queue order/iteration:
  gcn(b) -> xbuild transposes(b+1) -> conv1(b-2) -> tail part
Per block:
  X = D A^T D (DVE/ACT scale, PE transpose, scaled evac) -> 3 GCN layers
  (per-graph fp32 matmuls, 4-graph PSUM groups) -> layer-3 sort key -> PE
  transpose -> top-40 via DVE max/max_index/match_replace (5 rounds of
  top-8, exact fp32, reference tie order) -> int16 index roundtrip through
  DRAM into ap_gather layout -> feature gathers + fp32r casts (GPSIMD) ->
  fp32r conv1 (320-col chunks).
Tail (maxpool -> fp32r conv2 -> partial MLP + per-part finish incl the
final y DMA) runs in per-graph-independent parts so the last 32-graph slab
splits around the final block's gather chain.
"""
import numpy as np
import concourse.bass as bass
import concourse.mybir as mybir
import concourse.tile as tile
from concourse.masks import make_identity

F32 = mybir.dt.float32
F32R = mybir.dt.float32r
BF16 = mybir.dt.bfloat16
I16 = mybir.dt.int16
U32 = mybir.dt.uint32
AF = mybir.ActivationFunctionType
ALU = mybir.AluOpType

NP_ = 100
K = 40
GS = 128  # graph slots in tail
BG = 16   # graphs per block
NH = BG * K  # 640 top-K slots per block

# fp32 weight blob column layout
BL_W0A = 0
BL_W0B = 128
BL_W1 = 256
BL_W2 = 384
BL_W3 = 512
BL_B0 = 513
BL_B1 = 514
BL_B2 = 515
BL_MB1 = 516
BL_MW2 = 517
BL_GBASE = 518
BL_CB1 = 519
BL_CB2 = 520
BF_COLS = 521         # main fp32 blob (everything except mW1)
BM_COLS = 16 * 128    # mW1 blob: [0:32, 128*t : 128*(t+1)]

# fp32r blob layout
BR_CW1C = 0           # [128, 0:48]
BR_CW1C3 = 48         # [0:1, 48:64]
BR_CW2 = 64           # [0:16, 64+32j : 96+32j]
BR_COLS = 64 + 5 * 32


def ceil16(n):
    return (n + 15) // 16 * 16


# ======================================================================
# host packing
# ======================================================================
def pack_core_inputs(x, z, counts_ds, weights):
    """x [N,128] f32, z [N] int, counts_ds [G,100,100] (counts[g,d,s]),
    weights: dict of the model params (numpy). Returns in_map for the kernel."""
    import ml_dtypes
    N = x.shape[0]
    m = {}
    m["xT"] = np.ascontiguousarray(x.T.astype(np.float32))
    # z-embedding rows (pure gather; float values copied bit-exactly)
    m["zeT"] = np.ascontiguousarray(
        weights["z_table"].astype(np.float32)[np.asarray(z).astype(np.int64)].T)
    # adjacency pre-laid-out in the on-chip at4 tile format:
    # Atw[d, g*100+s] = counts[g, d, s] -> per-block DMA is 100 contiguous
    # descriptors instead of 1600 (SWDGE-ring friendly)
    m["Atw"] = np.ascontiguousarray(
        counts_ds.transpose(1, 0, 2).reshape(NP_, N).astype(ml_dtypes.bfloat16))
    deg = counts_ds.sum(axis=2).astype(np.float32)  # [G, 100]
    m["degT"] = np.ascontiguousarray(deg.T)         # [100, G]

    bf = np.zeros((128, BF_COLS), np.float32)
    W0 = weights["W0"].astype(np.float32)
    bf[:, BL_W0A:BL_W0A + 128] = W0[:128]
    bf[:, BL_W0B:BL_W0B + 128] = W0[128:]
    bf[:, BL_W1:BL_W1 + 128] = weights["W1"].astype(np.float32)
    bf[:, BL_W2:BL_W2 + 128] = weights["W2"].astype(np.float32)
    bf[:, BL_W3] = weights["W3"].astype(np.float32).reshape(128)
    bf[:, BL_B0] = weights["b0"].astype(np.float32)
    bf[:, BL_B1] = weights["b1"].astype(np.float32)
    bf[:, BL_B2] = weights["b2"].astype(np.float32)
    bf[:, BL_MB1] = weights["mb1"].astype(np.float32)
    bf[:, BL_MW2] = weights["mW2"].astype(np.float32).reshape(128)
    bf[:16, BL_GBASE] = np.arange(16) * NP_
    bf[:16, BL_CB1] = weights["cb1"].astype(np.float32)
    bf[:32, BL_CB2] = weights["cb2"].astype(np.float32)
    m["blobf"] = bf
    bm = np.zeros((32, BM_COLS), np.float32)
    mW1 = weights["mW1"].astype(np.float32)  # [512, 128], row index = o*16+t
    for t in range(16):
        bm[:, 128 * t: 128 * t + 128] = mW1[np.arange(32) * 16 + t]
    m["blobm"] = bm

    br = np.zeros((128, BR_COLS), np.float32)
    cw1 = weights["cw1"].astype(np.float32)[:, 0, :]  # [16, 385]
    for c in range(3):
        br[:, BR_CW1C + 16 * c: BR_CW1C + 16 * c + 16] = \
            cw1[:, 128 * c:128 * (c + 1)].T
    br[0, BR_CW1C3:BR_CW1C3 + 16] = cw1[:, 384]
    cw2 = weights["cw2"].astype(np.float32)  # [32, 16, 5]
    for j in range(5):
        br[:16, BR_CW2 + 32 * j: BR_CW2 + 32 * j + 32] = cw2[:, :, j].T
    m["blobr"] = br
    return m, float(weights["b3"][0]), float(weights["mb2"][0])


# ======================================================================
# kernel builder
# ======================================================================
class _Ctx:
    pass


def build_kernel(nc, G, b3_val, mb2_val):
    N = G * NP_
    dt = {}
    def din(name, shape, dtype=F32):
        dt[name] = nc.dram_tensor(name, shape, dtype, kind="ExternalInput")
        return dt[name]

    din("xT", [128, N])
    din("zeT", [128, N])
    din("Atw", [NP_, N], BF16)
    din("degT", [NP_, G])
    din("blobf", [128, BF_COLS])
    din("blobm", [32, BM_COLS])
    din("blobr", [128, BR_COLS])
    yout = nc.dram_tensor("y", [1, GS], F32, kind="ExternalOutput")

    with tile.TileContext(nc) as tc:
        with tc.tile_pool(name="wp", bufs=1) as wp, \
             tc.tile_pool(name="persist", bufs=1) as pp, \
             tc.tile_pool(name="dscr", bufs=2, space="DRAM") as dp:
            # degT first (prologue dinv chain is on the early critical path)
            degA = pp.tile([NP_, 128], F32, tag="degA")
            nc.sync.dma_start(out=degA[:, :G], in_=dt["degT"][:])
            blobf = wp.tile([128, BF_COLS], F32)
            blobm = wp.tile([32, BM_COLS], F32)
            blobr = wp.tile([128, BR_COLS], F32R)

            W = {}
            W["W0a"] = blobf[:, BL_W0A:BL_W0A + 128]
            W["W0b"] = blobf[:, BL_W0B:BL_W0B + 128]
            W["W1"] = blobf[:, BL_W1:BL_W1 + 128]
            W["W2"] = blobf[:, BL_W2:BL_W2 + 128]
            W["w3"] = blobf[:, BL_W3:BL_W3 + 1]
            W["b0"] = blobf[:, BL_B0:BL_B0 + 1]
            W["b1"] = blobf[:, BL_B1:BL_B1 + 1]
            W["b2"] = blobf[:, BL_B2:BL_B2 + 1]
            W["mb1"] = blobf[:, BL_MB1:BL_MB1 + 1]
            W["mW2"] = blobf[:, BL_MW2:BL_MW2 + 1]
            W["gbase"] = blobf[:16, BL_GBASE:BL_GBASE + 1]
            W["cb1"] = blobf[:16, BL_CB1:BL_CB1 + 1]
            W["cb2"] = blobf[:32, BL_CB2:BL_CB2 + 1]
            for t_ in range(16):
                W[f"mW1s_{t_}"] = blobm[:, 128 * t_: 128 * t_ + 128]
            W["cw1c"] = blobr[:, BR_CW1C:BR_CW1C + 48]
            W["cw1c3"] = blobr[:1, BR_CW1C3:BR_CW1C3 + 16]
            for j in range(5):
                W[f"cw2j_{j}"] = blobr[:16, BR_CW2 + 32 * j:BR_CW2 + 32 * j + 32]

            ident = wp.tile([128, 128], F32)
            make_identity(nc, ident[:])

            # ---- prologue: dinv for all graphs (one Sqrt table switch) ----
            dinvA = pp.tile([NP_, 128], F32, tag="dinvA")
            dmask = pp.tile([NP_, 128], F32, tag="dmask")
            nc.vector.tensor_scalar(dmask[:, :G], degA[:, :G], 0.5, None, op0=ALU.is_ge)
            nc.vector.tensor_scalar_max(dinvA[:, :G], degA[:, :G], 1.0)
            nc.vector.reciprocal(dinvA[:, :G], dinvA[:, :G])
            nc.scalar.activation(dinvA[:, :G], dinvA[:, :G], AF.Sqrt)
            nc.vector.tensor_mul(dinvA[:, :G], dinvA[:, :G], dmask[:, :G])

            # Y1 persistent [16, GS*K]; only pad graph slots need zeroing
            Y1 = pp.tile([16, GS * K], F32, tag="Y1")
            if G * K < GS * K:
                nc.vector.memset(Y1[:, G * K:], 0.0)

            with tc.tile_pool(name="blk", bufs=2) as blk, \
                 tc.tile_pool(name="ld", bufs=4) as ld, \
                 tc.tile_pool(name="sm", bufs=2) as sm, \
                 tc.tile_pool(name="grp", bufs=3) as grp, \
                 tc.tile_pool(name="gth", bufs=3) as gth, \
                 tc.tile_pool(name="gta", bufs=1) as gta, \
                 tc.tile_pool(name="st", bufs=2) as st, \
                 tc.tile_pool(name="pt", bufs=2, space="PSUM") as ptp, \
                 tc.tile_pool(name="phw", bufs=2, space="PSUM") as phw, \
                 tc.tile_pool(name="pag", bufs=2, space="PSUM") as pag, \
                 tc.tile_pool(name="pms", bufs=1, space="PSUM") as pms:
                cx = _Ctx()
                cx.nc, cx.tc, cx.dt, cx.W, cx.ident, cx.Y1, cx.dp = \
                    nc, tc, dt, W, ident, Y1, dp
                cx.blk, cx.sm, cx.grp, cx.gth, cx.st = blk, sm, grp, gth, st
                cx.ld = ld
                cx.gta = gta
                cx.ptp, cx.phw, cx.pag, cx.pms = ptp, phw, pag, pms
                cx.dinvA = dinvA
                cx.b3, cx.mb2 = float(b3_val), float(mb2_val)
                cx.zps = pms.tile([128, 128], F32, tag="zps")
                cx.yout = yout

                # block sizes: full 16-graph blocks, remainder split into two
                # small sub-blocks so the final drain chain covers few graphs
                bcnts = [BG] * (G // BG)
                rem = G - BG * (G // BG)
                if rem > 8:
                    bcnts += [8, rem - 8]
                elif rem > 0:
                    bcnts += [rem]
                nblk = len(bcnts)
                g0s = [sum(bcnts[:b]) for b in range(nblk)]
                cx.state = [dict() for _ in range(nblk)]
                nchunks = (G + 31) // 32

                # block-0 loads on the SP/HWDGE path, interleaved with the
                # weight blobs in first-use order (the DMA engines serialize
                # transfers, so issue order = arrival order): at4 gates the
                # X build, zT + W0a gate the first L0 matmul, xblk its
                # accumulate pass; blobr/blobm are needed much later.
                ncols0 = bcnts[0] * NP_
                at40 = cx.ld.tile([NP_, BG * NP_], BF16, tag="at4")
                nc.sync.dma_start(out=at40[:, :ncols0], in_=dt["Atw"][:, :ncols0])
                zT0 = cx.ld.tile([128, BG * NP_], F32, tag="zT")
                nc.sync.dma_start(out=zT0[:, :ncols0], in_=dt["zeT"][:, :ncols0])
                nc.sync.dma_start(out=blobf[:], in_=dt["blobf"][:])
                xblk0 = cx.ld.tile([128, BG * NP_], F32, tag="xblk")
                nc.sync.dma_start(out=xblk0[:, :ncols0], in_=dt["xT"][:, :ncols0])
                nc.sync.dma_start(out=blobr[:], in_=dt["blobr"][:].bitcast(F32R))
                nc.sync.dma_start(out=blobm[:], in_=dt["blobm"][:])
                cx.state[0]["at4"] = at40
                cx.state[0]["zT"] = zT0
                cx.state[0]["xblk"] = xblk0
                _emit_xbuild(cx, 0, g0s[0], bcnts[0])
                done_topk = 0
                next_chunk = 0
                for it in range(nblk):
                    if it == 0 and 1 < nblk:
                        _emit_loads(cx, 1, g0s[1], bcnts[1])
                    # block it+1's loads are emitted mid-GCN so their (DMA-
                    # engine-serialized) transfers land between the per-block
                    # index-roundtrip windows
                    if it >= 1 and it + 1 < nblk:
                        mid = (lambda j: (lambda: _emit_loads(
                            cx, j, g0s[j], bcnts[j])))(it + 1)
                    else:
                        mid = None
                    _emit_gcn(cx, it, bcnts[it], last=(it == nblk - 1), mid=mid)
                    if it + 1 < nblk:
                        _emit_xbuild(cx, it + 1, g0s[it + 1], bcnts[it + 1])
                    if it >= 2:
                        _emit_conv1(cx, it - 2, g0s[it - 2], bcnts[it - 2])
                        done_topk += bcnts[it - 2]
                        while (next_chunk < nchunks
                               and done_topk >= min(32 * (next_chunk + 1), G)):
                            _emit_tail_chunk(cx, next_chunk)
                            next_chunk += 1
                    if it >= 1:
                        _emit_gath(cx, it - 1, bcnts[it - 1])
                # final: gathers for the last block start (they wait on its
                # index roundtrip); meanwhile PE does block nblk-2's conv1 and
                # the ready part of the last tail slab, then the tiny rest.
                _emit_gath(cx, nblk - 1, bcnts[nblk - 1], last=True)
                _emit_conv1(cx, nblk - 2, g0s[nblk - 2], bcnts[nblk - 2])
                done_topk += bcnts[nblk - 2]
                while (next_chunk < nchunks
                       and done_topk >= min(32 * (next_chunk + 1), G)):
                    _emit_tail_chunk(cx, next_chunk)
                    next_chunk += 1
                cA = 32 * next_chunk
                partA = done_topk - cA
                if next_chunk < nchunks and partA > 0:
                    _emit_tail_chunk(cx, next_chunk, 0, partA)
                _emit_conv1(cx, nblk - 1, g0s[nblk - 1], bcnts[nblk - 1])
                done_topk += bcnts[nblk - 1]
                if next_chunk < nchunks:
                    rest = min(32, GS - cA) - partA
                    _emit_tail_chunk(cx, next_chunk, partA, rest)
                    next_chunk += 1
    return yout


def _emit_loads(cx, b, g0, bcnt, eng=None):
    """Issue block b's big DMAs on the Pool SWDGE path (bypasses HWDGE)."""
    nc = cx.nc
    eng = eng or nc.gpsimd
    n0 = g0 * NP_
    ncols = bcnt * NP_
    at4 = cx.ld.tile([NP_, BG * NP_], BF16, tag="at4")
    eng.dma_start(out=at4[:, :ncols], in_=cx.dt["Atw"][:, n0:n0 + ncols])
    zT = cx.ld.tile([128, BG * NP_], F32, tag="zT")
    eng.dma_start(out=zT[:, :ncols], in_=cx.dt["zeT"][:, n0:n0 + ncols])
    xblk = cx.ld.tile([128, BG * NP_], F32, tag="xblk")
    eng.dma_start(out=xblk[:, :ncols], in_=cx.dt["xT"][:, n0:n0 + ncols])
    cx.state[b]["at4"] = at4
    cx.state[b]["zT"] = zT
    cx.state[b]["xblk"] = xblk


def _emit_xbuild(cx, b, g0, bcnt):
    """X = D A^T D for block b."""
    nc = cx.nc
    s = cx.state[b]
    at4 = s["at4"]

    Xall = cx.blk.tile([NP_, BG * NP_], F32, tag="Xall")
    for g in range(bcnt):
        dcol = cx.dinvA[:, g0 + g:g0 + g + 1]
        bds = cx.grp.tile([NP_, NP_], F32, tag="bds")
        if g % 2 == 0:
            nc.vector.tensor_scalar_mul(bds[:], at4[:, g * NP_:(g + 1) * NP_], dcol)
        else:
            nc.scalar.activation(bds[:], at4[:, g * NP_:(g + 1) * NP_],
                                 AF.Copy, scale=dcol)
        pt = cx.ptp.tile([128, 128], F32, tag="ptrans")
        nc.tensor.transpose(pt[:NP_, :NP_], bds[:], cx.ident[:NP_, :NP_])
        if g % 2 == 0:
            nc.scalar.activation(Xall[:, g * NP_:(g + 1) * NP_], pt[:NP_, :NP_],
                                 AF.Copy, scale=dcol)
        else:
            nc.vector.tensor_scalar_mul(Xall[:, g * NP_:(g + 1) * NP_],
                                        pt[:NP_, :NP_], dcol)
    s["Xall"] = Xall


def _emit_gcn(cx, b, bcnt, last=False, mid=None):
    """GCN layers + sort keys + top-40 + index roundtrip for block b."""
    nc, W = cx.nc, cx.W
    s = cx.state[b]
    Xall, zT, xblk = s["Xall"], s["zT"], s["xblk"]

    h1 = cx.blk.tile([128, BG * NP_], F32, tag="h1")
    h2 = cx.blk.tile([128, BG * NP_], F32, tag="h2")
    h3 = cx.blk.tile([128, BG * NP_], F32, tag="h3")
    hs = [h1, h2, h3]
    s["hs"] = hs
    for layer in range(3):
        Wl = [None, W["W1"], W["W2"]][layer]
        bl = [W["b0"], W["b1"], W["b2"]][layer]
        if layer == 1 and mid is not None:
            mid()
        for g4 in range(0, bcnt, 4):
            gcnt = min(4, bcnt - g4)
            hwp = cx.phw.tile([128, 512], F32, tag="hw4")
            for i in range(gcnt):
                g = g4 + i
                sl = slice(g * NP_, (g + 1) * NP_)
                osl = slice(i * 128, i * 128 + 128)
                if layer == 0:
                    nc.tensor.matmul(hwp[:NP_, osl], lhsT=zT[:, sl],
                                     rhs=W["W0a"], start=True, stop=False)
                    nc.tensor.matmul(hwp[:NP_, osl], lhsT=xblk[:, sl],
                                     rhs=W["W0b"], start=False, stop=True)
                else:
                    nc.tensor.matmul(hwp[:NP_, osl], lhsT=hs[layer - 1][:, sl],
                                     rhs=Wl, start=True, stop=True)
            P4 = cx.grp.tile([128, 512], F32, tag="P4")
            if layer == 1:
                nc.scalar.activation(P4[:NP_, :gcnt * 128], hwp[:NP_, :gcnt * 128],
                                     AF.Copy)
            else:
                nc.vector.tensor_copy(P4[:NP_, :gcnt * 128], hwp[:NP_, :gcnt * 128])
            agg = cx.pag.tile([128, 512], F32, tag="agg")
            for i in range(gcnt):
                g = g4 + i
                nc.tensor.matmul(agg[:, i * NP_:(i + 1) * NP_],
                                 lhsT=P4[:NP_, i * 128:(i + 1) * 128],
                                 rhs=Xall[:, g * NP_:(g + 1) * NP_],
                                 start=True, stop=True)
            nc.scalar.activation(hs[layer][:, g4 * NP_: (g4 + gcnt) * NP_],
                                 agg[:, :gcnt * NP_], AF.Tanh, bias=bl)

    # ---- layer 3: per-node sort key (pre-tanh) ----
    vps = cx.phw.tile([128, 512], F32, tag="hw4")
    for i in range(bcnt):
        nc.tensor.matmul(vps[:NP_, i:i + 1], lhsT=h3[:, i * NP_:(i + 1) * NP_],
                         rhs=W["w3"], start=True, stop=True)
    vsb = cx.sm.tile([NP_, BG], F32, tag="vsb")
    nc.vector.tensor_copy(vsb[:, :bcnt], vps[:NP_, :bcnt])
    h4ps = cx.pag.tile([128, 512], F32, tag="agg")
    for i in range(bcnt):
        nc.tensor.matmul(h4ps[:NP_, i:i + 1], lhsT=Xall[:, i * NP_:(i + 1) * NP_],
                         rhs=vsb[:, i:i + 1], start=True, stop=True)
    h4blk = cx.sm.tile([NP_, BG], F32, tag="h4blk")
    nc.scalar.activation(h4blk[:, :bcnt], h4ps[:NP_, :bcnt], AF.Copy, bias=cx.b3)

    # ---- transpose keys to [graphs, nodes], top-40 via 5x top-8 ----
    h4Tps = cx.ptp.tile([128, 128], F32, tag="ptrans")
    nc.tensor.transpose(h4Tps[:bcnt, :NP_], h4blk[:, :bcnt], cx.ident[:NP_, :NP_])
    h4T = cx.sm.tile([BG, NP_], F32, tag="h4T")
    nc.vector.tensor_copy(h4T[:bcnt, :], h4Tps[:bcnt, :NP_])

    vals = cx.sm.tile([BG, K], F32, tag="vals")
    idxs32 = cx.sm.tile([BG, K], U32, tag="idxs32")
    idx16 = cx.sm.tile([BG, K], I16, tag="idx16")
    idxf = cx.sm.tile([BG, K], F32, tag="idxf")
    scr_idx = cx.dp.tile([BG, K], I16, tag="scr_idx")
    if bcnt < BG:
        nc.vector.memset(vals[:], 0.0)
        nc.vector.memset(idx16[:], 0)
    for r in range(5):
        nc.vector.max(vals[:bcnt, 8 * r:8 * r + 8], h4T[:bcnt, :])
        nc.vector.max_index(idxs32[:bcnt, 8 * r:8 * r + 8],
                            vals[:bcnt, 8 * r:8 * r + 8], h4T[:bcnt, :])
        if r < 4:
            nc.vector.match_replace(h4T[:bcnt, :], vals[:bcnt, 8 * r:8 * r + 8],
                                    h4T[:bcnt, :], -1e30)
        if last:
            # final block: convert + write this round's 8 index columns
            # immediately so the scratch-write latency hides under the rounds
            sl8 = slice(8 * r, 8 * r + 8)
            nc.vector.tensor_copy(idxf[:bcnt, sl8], idxs32[:bcnt, sl8])
            nc.vector.tensor_scalar_add(idxf[:bcnt, sl8], idxf[:bcnt, sl8],
                                        W["gbase"][:bcnt, :])
            nc.vector.tensor_copy(idx16[:bcnt, sl8], idxf[:bcnt, sl8])
            nc.sync.dma_start(out=scr_idx[:, sl8], in_=idx16[:, sl8])

    if not last:
        nc.vector.tensor_copy(idxf[:bcnt, :], idxs32[:bcnt, :])
        nc.vector.tensor_scalar_add(idxf[:bcnt, :], idxf[:bcnt, :],
                                    W["gbase"][:bcnt, :])
        nc.vector.tensor_copy(idx16[:bcnt, :], idxf[:bcnt, :])

    # ---- roundtrip through DRAM: wrapped idx layout + flat vals ----
    if not last:
        nc.sync.dma_start(out=scr_idx[:], in_=idx16[:])
    iw128 = cx.sm.tile([128, NH // 16], I16, tag="iw128")
    wrapped = scr_idx[:].rearrange("g k -> (g k)").rearrange("(c p) -> p c", p=16)
    for c in range(8):
        eng = nc.gpsimd if (last and c >= 5) else nc.sync
        eng.dma_start(out=iw128[16 * c:16 * (c + 1), :], in_=wrapped)

    nc.scalar.activation(vals[:bcnt, :], vals[:bcnt, :], AF.Tanh)
    scr_val = cx.dp.tile([BG, K], F32, tag="scr_val")
    nc.sync.dma_start(out=scr_val[:], in_=vals[:])
    h4row = cx.ld.tile([1, NH], F32R, tag="h4row")
    nc.sync.dma_start(out=h4row[:],
                      in_=scr_val[:].rearrange("g k -> (g k)").rearrange(
                          "(o n) -> o n", o=1).bitcast(F32R))
    s["iw128"], s["h4row"] = iw128, h4row


def _emit_gath(cx, b, bcnt, last=False):
    """Feature gathers for block b (Pool; consumed by conv1 a block later)."""
    nc = cx.nc
    s = cx.state[b]
    ncols = bcnt * NP_
    nki = ceil16(bcnt * K)
    grs = []
    for c in range(3):
        gat = cx.gta.tile([128, NH], F32, tag=f"gath{c}")
        nc.gpsimd.ap_gather(gat[:, :nki], s["hs"][c][:, :ncols],
                            s["iw128"][:, :nki // 16],
                            channels=128, num_elems=ncols, d=1, num_idxs=nki)
        gr = cx.gth.tile([128, NH], F32R, tag=f"gr{c}")
        # final block: DVE is idle at the drain, so the f32r casts pipeline
        # against the next gather instead of serializing on Pool
        ceng = nc.vector if last else nc.gpsimd
        ceng.tensor_copy(gr[:, :nki], gat[:, :nki])
        grs.append(gr)
    s["grs"] = grs


def _emit_conv1(cx, b, g0, bcnt):
    """conv1 for block b (gathers ran a block earlier -> PE never stalls)."""
    nc, W = cx.nc, cx.W
    s = cx.state[b]
    grs = s["grs"]
    nk = bcnt * K
    hof = g0 * K
    for ch0 in range(0, nk, 320):
        cw = min(320, nk - ch0)
        cps = cx.phw.tile([128, 512], F32, tag="hw4")
        for c in range(3):
            nc.tensor.matmul(cps[:16, :cw], lhsT=W["cw1c"][:, 16 * c:16 * (c + 1)],
                             rhs=grs[c][:, ch0:ch0 + cw], start=(c == 0), stop=False)
        nc.tensor.matmul(cps[:16, :cw], lhsT=W["cw1c3"],
                         rhs=s["h4row"][:1, ch0:ch0 + cw], start=False, stop=True)
        nc.scalar.activation(cx.Y1[:, hof + ch0: hof + ch0 + cw],
                             cps[:16, :cw], AF.Relu, bias=W["cb1"])


def _emit_tail_chunk(cx, ch, goff=0, gcnt=32):
    """Maxpool + conv2 + partial mlp1 + finish for graph slots
    [32*ch+goff, 32*ch+goff+gcnt) (per-graph independent -> splittable)."""
    nc, W = cx.nc, cx.W
    g0 = 32 * ch + goff
    Y2c = cx.st.tile([16, 640], F32R, tag="Y2c")
    y1v = cx.Y1[:, 40 * g0:40 * (g0 + gcnt)].rearrange(
        "p (m two) -> p m two", two=2)
    nc.vector.tensor_tensor(Y2c[:, :20 * gcnt], y1v[:, :, 0], y1v[:, :, 1],
                            op=ALU.max)

    # conv2: 5 shifted matmuls over the part's graph slots; pad tiny parts
    # to 16 slots so the fp32r matmul hits the >=256-free-dim fast path
    # (padded slots read stale-but-finite Y2c cols and are never consumed)
    gmm = max(gcnt, 16)
    cps = cx.pms.tile([32, 512], F32, tag="cps")
    for j in range(5):
        rhs = Y2c[:, :20 * gmm].rearrange("p (g t) -> p g t", t=20)[:, :, j:j + 16]
        nc.tensor.matmul(cps[:, :16 * gmm], lhsT=W[f"cw2j_{j}"], rhs=rhs,
                         start=(j == 0), stop=(j == 4))
    Y3c = cx.st.tile([32, 512], F32, tag="Y3c")
    nc.scalar.activation(Y3c[:, :16 * gcnt], cps[:, :16 * gcnt],
                         AF.Relu, bias=W["cb2"])

    # mlp1 partial: accumulate this part's columns of zps
    for t in range(16):
        rhs = Y3c[:, :16 * gcnt].rearrange("p (g t) -> p g t", t=16)[:, :, t]
        nc.tensor.matmul(cx.zps[:, g0:g0 + gcnt],
                         lhsT=W[f"mW1s_{t}"], rhs=rhs,
                         start=(t == 0), stop=(t == 15))

    # finish this part's graphs: relu -> mW2 -> +mb2 -> DMA out
    Z1 = cx.st.tile([128, 32], F32, tag="Z1")
    nc.scalar.activation(Z1[:, :gcnt], cx.zps[:, g0:g0 + gcnt],
                         AF.Relu, bias=W["mb1"])
    ypst = cx.pms.tile([32, 512], F32, tag="cps")
    nc.tensor.matmul(ypst[:1, :gcnt], lhsT=W["mW2"], rhs=Z1[:, :gcnt],
                     start=True, stop=True)
    ysb = cx.st.tile([1, 32], F32, tag="ysb")
    nc.scalar.activation(ysb[:, :gcnt], ypst[:1, :gcnt], AF.Copy, bias=cx.mb2)
    nc.sync.dma_start(out=cx.yout[:1, g0:g0 + gcnt], in_=ysb[:, :gcnt])


# ======================================================================
# public entry point: kernel(**inputs) -> np.ndarray [1000, 1]
# ======================================================================
NUM_CORES = 8
NUM_GRAPHS = 1000
G_PER_CORE = NUM_GRAPHS // NUM_CORES  # 125

_cache = {}


def _get_compiled(b3_val, mb2_val):
    key = "nc"
    if key not in _cache:
        from concourse import bacc
        nc = bacc.Bacc("TRN2", target_bir_lowering=False, debug=False,
                       num_devices=NUM_CORES, dynamic_dma_scratch_size=16384)
        build_kernel(nc, G_PER_CORE, b3_val, mb2_val)
        nc.compile()
        _cache[key] = nc
    return _cache[key]


def kernel(x, z, edge_index, batch, z_table, W0, b0, W1, b1, W2, b2, W3, b3,
           cw1, cb1, cw2, cb2, mW1, mb1, mW2, mb2, **_unused):
    x = np.asarray(x, np.float32)
    z = np.asarray(z)
    ei = np.asarray(edge_index)
    weights = dict(z_table=np.asarray(z_table), W0=np.asarray(W0),
                   b0=np.asarray(b0), W1=np.asarray(W1), b1=np.asarray(b1),
                   W2=np.asarray(W2), b2=np.asarray(b2), W3=np.asarray(W3),
                   b3=np.asarray(b3), cw1=np.asarray(cw1), cb1=np.asarray(cb1),
                   cw2=np.asarray(cw2), cb2=np.asarray(cb2), mW1=np.asarray(mW1),
                   mb1=np.asarray(mb1), mW2=np.asarray(mW2), mb2=np.asarray(mb2))

    # dense per-graph adjacency counts: row (g, d), col s  (edges are intra-graph)
    src = ei[0].astype(np.int64)
    dst = ei[1].astype(np.int64)
    code = dst * NP_ + (src % NP_)  # == (g*100 + d)*100 + s
    counts = np.bincount(code, minlength=NUM_GRAPHS * NP_ * NP_).astype(np.float32)
    counts = counts.reshape(NUM_GRAPHS, NP_, NP_)

    b3v = float(np.asarray(b3).reshape(-1)[0])
    mb2v = float(np.asarray(mb2).reshape(-1)[0])
    nc = _get_compiled(b3v, mb2v)

    in_maps = []
    for c in range(NUM_CORES):
        g0 = c * G_PER_CORE
        n0 = g0 * NP_
        n1 = n0 + G_PER_CORE * NP_
        m, _, _ = pack_core_inputs(x[n0:n1], z[n0:n1], counts[g0:g0 + G_PER_CORE],
                                   weights)
        in_maps.append(m)

    from concourse.bass_utils import run_bass_kernel_spmd
    res = run_bass_kernel_spmd(nc, in_maps, list(range(NUM_CORES)))
    y = np.concatenate([res.results[c]["y"][0, :G_PER_CORE]
                        for c in range(NUM_CORES)])
    return y.reshape(NUM_GRAPHS, 1).astype(np.float32)


# revision 57
# speedup vs baseline: 1.0203x; 1.0120x over previous
"""Trainium2 Bass kernel for the DGCNN (gnn_message_passing) problem.

Strategy: data-parallel over the graph batch — 125 graphs per NeuronCore
(8 cores). Host does only layout/integer work: shards/transposes inputs,
converts the COO edge list into per-graph dense adjacency count matrices and
their integer row-sums (degrees), gathers the z-embedding rows (pure layout),
and packs the small weights into two SBUF-resident blobs (one fp32, one
fp32r) loaded with a single DMA each. All model FLOAT math (degree
normalization, GCN layers, sort-pooling, convolutions, MLP) runs on device:
fp32 for everything feeding the sort keys (sort-pool ordering is numerically
fragile), fp32r only for post-sort convolutions.

Per-core schedule: 7 blocks of 16 graphs + tail blocks of 8 and 5 graphs
(small tail blocks shorten the drain), software-pipelined two stages deep:
block b's GCN matmuls (PE) overlap block b-1's top-K index roundtrip +
feature gathers (SP-DMA/Pool) and block b-2's conv1 (PE) — conv1 is
deferred two blocks so its gathered inputs are always long ready and PE
never stalls on the gather chain. Input loads are issued mid-GCN on the
Pool SWDGE path (zT/xblk) and pre-laid-out on host so each is few-
descriptor; weights arrive as three blob DMAs. PE queue order/iteration:
  gcn(b) -> xbuild transposes(b+1) -> conv1(b-2) -> tail part
Per block:
  X = D A^T D (DVE/ACT scale, PE transpose, scaled evac) -> 3 GCN layers
  (per-graph fp32 matmuls, 4-graph PSUM groups) -> layer-3 sort key -> PE
  transpose -> top-40 via DVE max/max_index/match_replace (5 rounds of
  top-8, exact fp32, reference tie order) -> int16 index roundtrip through
  DRAM into ap_gather layout -> feature gathers + fp32r casts (GPSIMD) ->
  fp32r conv1 (320-col chunks).
Tail (maxpool -> fp32r conv2 -> partial MLP + per-part finish incl the
final y DMA) runs in per-graph-independent parts so the last 32-graph slab
splits around the final block's gather chain.
"""
import numpy as np
import concourse.bass as bass
import concourse.mybir as mybir
import concourse.tile as tile
from concourse.masks import make_identity

F32 = mybir.dt.float32
F32R = mybir.dt.float32r
BF16 = mybir.dt.bfloat16
I16 = mybir.dt.int16
U32 = mybir.dt.uint32
AF = mybir.ActivationFunctionType
ALU = mybir.AluOpType

NP_ = 100
K = 40
GS = 128  # graph slots in tail
BG = 16   # graphs per block
NH = BG * K  # 640 top-K slots per block

# fp32 weight blob column layout
BL_W0A = 0
BL_W0B = 128
BL_W1 = 256
BL_W2 = 384
BL_W3 = 512
BL_B0 = 513
BL_B1 = 514
BL_B2 = 515
BL_MB1 = 516
BL_MW2 = 517
BL_GBASE = 518
BL_CB1 = 519
BL_CB2 = 520
BF_COLS = 521         # main fp32 blob (everything except mW1)
BM_COLS = 16 * 128    # mW1 blob: [0:32, 128*t : 128*(t+1)]

# fp32r blob layout
BR_CW1C = 0           # [128, 0:48]
BR_CW1C3 = 48         # [0:1, 48:64]
BR_CW2 = 64           # [0:16, 64+32j : 96+32j]
BR_COLS = 64 + 5 * 32


def ceil16(n):
    return (n + 15) // 16 * 16


# ======================================================================
# host packing
# ======================================================================
def pack_core_inputs(x, z, counts_ds, weights):
    """x [N,128] f32, z [N] int, counts_ds [G,100,100] (counts[g,d,s]),
    weights: dict of the model params (numpy). Returns in_map for the kernel."""
    import ml_dtypes
    N = x.shape[0]
    m = {}
    m["xT"] = np.ascontiguousarray(x.T.astype(np.float32))
    # z-embedding rows (pure gather; float values copied bit-exactly)
    m["zeT"] = np.ascontiguousarray(
        weights["z_table"].astype(np.float32)[np.asarray(z).astype(np.int64)].T)
    # adjacency pre-laid-out in the on-chip at4 tile format:
    # Atw[d, g*100+s] = counts[g, d, s] -> per-block DMA is 100 contiguous
    # descriptors instead of 1600 (SWDGE-ring friendly)
    m["Atw"] = np.ascontiguousarray(
        counts_ds.transpose(1, 0, 2).reshape(NP_, N).astype(ml_dtypes.bfloat16))
    deg = counts_ds.sum(axis=2).astype(np.float32)  # [G, 100]
    m["degT"] = np.ascontiguousarray(deg.T)         # [100, G]

    bf = np.zeros((128, BF_COLS), np.float32)
    W0 = weights["W0"].astype(np.float32)
    bf[:, BL_W0A:BL_W0A + 128] = W0[:128]
    bf[:, BL_W0B:BL_W0B + 128] = W0[128:]
    bf[:, BL_W1:BL_W1 + 128] = weights["W1"].astype(np.float32)
    bf[:, BL_W2:BL_W2 + 128] = weights["W2"].astype(np.float32)
    bf[:, BL_W3] = weights["W3"].astype(np.float32).reshape(128)
    bf[:, BL_B0] = weights["b0"].astype(np.float32)
    bf[:, BL_B1] = weights["b1"].astype(np.float32)
    bf[:, BL_B2] = weights["b2"].astype(np.float32)
    bf[:, BL_MB1] = weights["mb1"].astype(np.float32)
    bf[:, BL_MW2] = weights["mW2"].astype(np.float32).reshape(128)
    bf[:16, BL_GBASE] = np.arange(16) * NP_
    bf[:16, BL_CB1] = weights["cb1"].astype(np.float32)
    bf[:32, BL_CB2] = weights["cb2"].astype(np.float32)
    m["blobf"] = bf
    bm = np.zeros((32, BM_COLS), np.float32)
    mW1 = weights["mW1"].astype(np.float32)  # [512, 128], row index = o*16+t
    for t in range(16):
        bm[:, 128 * t: 128 * t + 128] = mW1[np.arange(32) * 16 + t]
    m["blobm"] = bm

    br = np.zeros((128, BR_COLS), np.float32)
    cw1 = weights["cw1"].astype(np.float32)[:, 0, :]  # [16, 385]
    for c in range(3):
        br[:, BR_CW1C + 16 * c: BR_CW1C + 16 * c + 16] = \
            cw1[:, 128 * c:128 * (c + 1)].T
    br[0, BR_CW1C3:BR_CW1C3 + 16] = cw1[:, 384]
    cw2 = weights["cw2"].astype(np.float32)  # [32, 16, 5]
    for j in range(5):
        br[:16, BR_CW2 + 32 * j: BR_CW2 + 32 * j + 32] = cw2[:, :, j].T
    m["blobr"] = br
    return m, float(weights["b3"][0]), float(weights["mb2"][0])


# ======================================================================
# kernel builder
# ======================================================================
class _Ctx:
    pass


def build_kernel(nc, G, b3_val, mb2_val):
    N = G * NP_
    dt = {}
    def din(name, shape, dtype=F32):
        dt[name] = nc.dram_tensor(name, shape, dtype, kind="ExternalInput")
        return dt[name]

    din("xT", [128, N])
    din("zeT", [128, N])
    din("Atw", [NP_, N], BF16)
    din("degT", [NP_, G])
    din("blobf", [128, BF_COLS])
    din("blobm", [32, BM_COLS])
    din("blobr", [128, BR_COLS])
    yout = nc.dram_tensor("y", [1, GS], F32, kind="ExternalOutput")

    with tile.TileContext(nc) as tc:
        with tc.tile_pool(name="wp", bufs=1) as wp, \
             tc.tile_pool(name="persist", bufs=1) as pp, \
             tc.tile_pool(name="dscr", bufs=2, space="DRAM") as dp:
            # degT first (prologue dinv chain is on the early critical path)
            degA = pp.tile([NP_, 128], F32, tag="degA")
            nc.sync.dma_start(out=degA[:, :G], in_=dt["degT"][:])
            blobf = wp.tile([128, BF_COLS], F32)
            blobm = wp.tile([32, BM_COLS], F32)
            blobr = wp.tile([128, BR_COLS], F32R)

            W = {}
            W["W0a"] = blobf[:, BL_W0A:BL_W0A + 128]
            W["W0b"] = blobf[:, BL_W0B:BL_W0B + 128]
            W["W1"] = blobf[:, BL_W1:BL_W1 + 128]
            W["W2"] = blobf[:, BL_W2:BL_W2 + 128]
            W["w3"] = blobf[:, BL_W3:BL_W3 + 1]
            W["b0"] = blobf[:, BL_B0:BL_B0 + 1]
            W["b1"] = blobf[:, BL_B1:BL_B1 + 1]
            W["b2"] = blobf[:, BL_B2:BL_B2 + 1]
            W["mb1"] = blobf[:, BL_MB1:BL_MB1 + 1]
            W["mW2"] = blobf[:, BL_MW2:BL_MW2 + 1]
            W["gbase"] = blobf[:16, BL_GBASE:BL_GBASE + 1]
            W["cb1"] = blobf[:16, BL_CB1:BL_CB1 + 1]
            W["cb2"] = blobf[:32, BL_CB2:BL_CB2 + 1]
            for t_ in range(16):
                W[f"mW1s_{t_}"] = blobm[:, 128 * t_: 128 * t_ + 128]
            W["cw1c"] = blobr[:, BR_CW1C:BR_CW1C + 48]
            W["cw1c3"] = blobr[:1, BR_CW1C3:BR_CW1C3 + 16]
            for j in range(5):
                W[f"cw2j_{j}"] = blobr[:16, BR_CW2 + 32 * j:BR_CW2 + 32 * j + 32]

            ident = wp.tile([128, 128], F32)
            make_identity(nc, ident[:])

            # ---- prologue: dinv for all graphs (one Sqrt table switch) ----
            dinvA = pp.tile([NP_, 128], F32, tag="dinvA")
            dmask = pp.tile([NP_, 128], F32, tag="dmask")
            nc.vector.tensor_scalar(dmask[:, :G], degA[:, :G], 0.5, None, op0=ALU.is_ge)
            nc.vector.tensor_scalar_max(dinvA[:, :G], degA[:, :G], 1.0)
            nc.vector.reciprocal(dinvA[:, :G], dinvA[:, :G])
            nc.scalar.activation(dinvA[:, :G], dinvA[:, :G], AF.Sqrt)
            nc.vector.tensor_mul(dinvA[:, :G], dinvA[:, :G], dmask[:, :G])

            # Y1 persistent [16, GS*K]; only pad graph slots need zeroing
            Y1 = pp.tile([16, GS * K], F32, tag="Y1")
            if G * K < GS * K:
                nc.vector.memset(Y1[:, G * K:], 0.0)

            with tc.tile_pool(name="blk", bufs=2) as blk, \
                 tc.tile_pool(name="ld", bufs=4) as ld, \
                 tc.tile_pool(name="sm", bufs=2) as sm, \
                 tc.tile_pool(name="grp", bufs=3) as grp, \
                 tc.tile_pool(name="gth", bufs=3) as gth, \
                 tc.tile_pool(name="gta", bufs=1) as gta, \
                 tc.tile_pool(name="st", bufs=2) as st, \
                 tc.tile_pool(name="pt", bufs=2, space="PSUM") as ptp, \
                 tc.tile_pool(name="phw", bufs=2, space="PSUM") as phw, \
                 tc.tile_pool(name="pag", bufs=2, space="PSUM") as pag, \
                 tc.tile_pool(name="pms", bufs=1, space="PSUM") as pms:
                cx = _Ctx()
                cx.nc, cx.tc, cx.dt, cx.W, cx.ident, cx.Y1, cx.dp = \
                    nc, tc, dt, W, ident, Y1, dp
                cx.blk, cx.sm, cx.grp, cx.gth, cx.st = blk, sm, grp, gth, st
                cx.ld = ld
                cx.gta = gta
                cx.ptp, cx.phw, cx.pag, cx.pms = ptp, phw, pag, pms
                cx.dinvA = dinvA
                cx.b3, cx.mb2 = float(b3_val), float(mb2_val)
                cx.zps = pms.tile([128, 128], F32, tag="zps")
                cx.yout = yout

                # block sizes: full 16-graph blocks, remainder split into two
                # small sub-blocks so the final drain chain covers few graphs
                bcnts = [BG] * (G // BG)
                rem = G - BG * (G // BG)
                if rem > 8:
                    bcnts += [8, rem - 8]
                elif rem > 0:
                    bcnts += [rem]
                nblk = len(bcnts)
                g0s = [sum(bcnts[:b]) for b in range(nblk)]
                cx.state = [dict() for _ in range(nblk)]
                nchunks = (G + 31) // 32

                # block-0 loads on the SP/HWDGE path, interleaved with the
                # weight blobs in first-use order (the DMA engines serialize
                # transfers, so issue order = arrival order): at4 gates the
                # X build, zT + W0a gate the first L0 matmul, xblk its
                # accumulate pass; blobr/blobm are needed much later.
                ncols0 = bcnts[0] * NP_
                at40 = cx.ld.tile([NP_, BG * NP_], BF16, tag="at4")
                nc.sync.dma_start(out=at40[:, :ncols0], in_=dt["Atw"][:, :ncols0])
                zT0 = cx.ld.tile([128, BG * NP_], F32, tag="zT")
                nc.sync.dma_start(out=zT0[:, :ncols0], in_=dt["zeT"][:, :ncols0])
                nc.sync.dma_start(out=blobf[:], in_=dt["blobf"][:])
                xblk0 = cx.ld.tile([128, BG * NP_], F32, tag="xblk")
                nc.sync.dma_start(out=xblk0[:, :ncols0], in_=dt["xT"][:, :ncols0])
                nc.sync.dma_start(out=blobr[:], in_=dt["blobr"][:].bitcast(F32R))
                nc.sync.dma_start(out=blobm[:], in_=dt["blobm"][:])
                cx.state[0]["at4"] = at40
                cx.state[0]["zT"] = zT0
                cx.state[0]["xblk"] = xblk0
                _emit_xbuild(cx, 0, g0s[0], bcnts[0])
                done_topk = 0
                next_chunk = 0
                for it in range(nblk):
                    if it == 0 and 1 < nblk:
                        _emit_loads(cx, 1, g0s[1], bcnts[1])
                    # block it+1's loads are emitted mid-GCN so their (DMA-
                    # engine-serialized) transfers land between the per-block
                    # index-roundtrip windows
                    if it >= 1 and it + 1 < nblk:
                        mid = (lambda j: (lambda: _emit_loads(
                            cx, j, g0s[j], bcnts[j])))(it + 1)
                    else:
                        mid = None
                    _emit_gcn(cx, it, bcnts[it], last=(it == nblk - 1), mid=mid)
                    if it + 1 < nblk:
                        _emit_xbuild(cx, it + 1, g0s[it + 1], bcnts[it + 1])
                    if it >= 2:
                        _emit_conv1(cx, it - 2, g0s[it - 2], bcnts[it - 2])
                        done_topk += bcnts[it - 2]
                        while (next_chunk < nchunks
                               and done_topk >= min(32 * (next_chunk + 1), G)):
                            _emit_tail_chunk(cx, next_chunk)
                            next_chunk += 1
                    if it >= 1:
                        _emit_gath(cx, it - 1, bcnts[it - 1])
                # final: gathers for the last block start (they wait on its
                # index roundtrip); meanwhile PE does block nblk-2's conv1 and
                # the ready part of the last tail slab, then the tiny rest.
                _emit_gath(cx, nblk - 1, bcnts[nblk - 1], last=True)
                _emit_conv1(cx, nblk - 2, g0s[nblk - 2], bcnts[nblk - 2])
                done_topk += bcnts[nblk - 2]
                while (next_chunk < nchunks
                       and done_topk >= min(32 * (next_chunk + 1), G)):
                    _emit_tail_chunk(cx, next_chunk)
                    next_chunk += 1
                cA = 32 * next_chunk
                partA = done_topk - cA
                if next_chunk < nchunks and partA > 0:
                    _emit_tail_chunk(cx, next_chunk, 0, partA)
                _emit_conv1(cx, nblk - 1, g0s[nblk - 1], bcnts[nblk - 1])
                done_topk += bcnts[nblk - 1]
                if next_chunk < nchunks:
                    rest = min(32, GS - cA) - partA
                    _emit_tail_chunk(cx, next_chunk, partA, rest)
                    next_chunk += 1
    return yout


def _emit_loads(cx, b, g0, bcnt, eng=None):
    """Issue block b's big DMAs on the Pool SWDGE path (bypasses HWDGE)."""
    nc = cx.nc
    eng = eng or nc.gpsimd
    n0 = g0 * NP_
    ncols = bcnt * NP_
    at4 = cx.ld.tile([NP_, BG * NP_], BF16, tag="at4")
    eng.dma_start(out=at4[:, :ncols], in_=cx.dt["Atw"][:, n0:n0 + ncols])
    zT = cx.ld.tile([128, BG * NP_], F32, tag="zT")
    eng.dma_start(out=zT[:, :ncols], in_=cx.dt["zeT"][:, n0:n0 + ncols])
    xblk = cx.ld.tile([128, BG * NP_], F32, tag="xblk")
    eng.dma_start(out=xblk[:, :ncols], in_=cx.dt["xT"][:, n0:n0 + ncols])
    cx.state[b]["at4"] = at4
    cx.state[b]["zT"] = zT
    cx.state[b]["xblk"] = xblk


def _emit_xbuild(cx, b, g0, bcnt):
    """X = D A^T D for block b."""
    nc = cx.nc
    s = cx.state[b]
    at4 = s["at4"]

    Xall = cx.blk.tile([NP_, BG * NP_], F32, tag="Xall")
    for g in range(bcnt):
        dcol = cx.dinvA[:, g0 + g:g0 + g + 1]
        bds = cx.grp.tile([NP_, NP_], F32, tag="bds")
        if g % 2 == 0:
            nc.vector.tensor_scalar_mul(bds[:], at4[:, g * NP_:(g + 1) * NP_], dcol)
        else:
            nc.scalar.activation(bds[:], at4[:, g * NP_:(g + 1) * NP_],
                                 AF.Copy, scale=dcol)
        pt = cx.ptp.tile([128, 128], F32, tag="ptrans")
        nc.tensor.transpose(pt[:NP_, :NP_], bds[:], cx.ident[:NP_, :NP_])
        if g % 2 == 0:
            nc.scalar.activation(Xall[:, g * NP_:(g + 1) * NP_], pt[:NP_, :NP_],
                                 AF.Copy, scale=dcol)
        else:
            nc.vector.tensor_scalar_mul(Xall[:, g * NP_:(g + 1) * NP_],
                                        pt[:NP_, :NP_], dcol)
    s["Xall"] = Xall


def _emit_gcn(cx, b, bcnt, last=False, mid=None):
    """GCN layers + sort keys + top-40 + index roundtrip for block b."""
    nc, W = cx.nc, cx.W
    s = cx.state[b]
    Xall, zT, xblk = s["Xall"], s["zT"], s["xblk"]

    h1 = cx.blk.tile([128, BG * NP_], F32, tag="h1")
    h2 = cx.blk.tile([128, BG * NP_], F32, tag="h2")
    h3 = cx.blk.tile([128, BG * NP_], F32, tag="h3")
    hs = [h1, h2, h3]
    s["hs"] = hs
    for layer in range(3):
        Wl = [None, W["W1"], W["W2"]][layer]
        bl = [W["b0"], W["b1"], W["b2"]][layer]
        if layer == 1 and mid is not None:
            mid()
        for g4 in range(0, bcnt, 4):
            gcnt = min(4, bcnt - g4)
            hwp = cx.phw.tile([128, 512], F32, tag="hw4")
            for i in range(gcnt):
                g = g4 + i
                sl = slice(g * NP_, (g + 1) * NP_)
                osl = slice(i * 128, i * 128 + 128)
                if layer == 0:
                    nc.tensor.matmul(hwp[:NP_, osl], lhsT=zT[:, sl],
                                     rhs=W["W0a"], start=True, stop=False)
                    nc.tensor.matmul(hwp[:NP_, osl], lhsT=xblk[:, sl],
                                     rhs=W["W0b"], start=False, stop=True)
                else:
                    nc.tensor.matmul(hwp[:NP_, osl], lhsT=hs[layer - 1][:, sl],
                                     rhs=Wl, start=True, stop=True)
            P4 = cx.grp.tile([128, 512], F32, tag="P4")
            if layer == 0:
                # ACT: at the block boundary DVE is still draining the
                # previous block's top-k rounds; ACT is free
                nc.scalar.activation(P4[:NP_, :gcnt * 128], hwp[:NP_, :gcnt * 128],
                                     AF.Copy)
            else:
                nc.vector.tensor_copy(P4[:NP_, :gcnt * 128], hwp[:NP_, :gcnt * 128])
            agg = cx.pag.tile([128, 512], F32, tag="agg")
            for i in range(gcnt):
                g = g4 + i
                nc.tensor.matmul(agg[:, i * NP_:(i + 1) * NP_],
                                 lhsT=P4[:NP_, i * 128:(i + 1) * 128],
                                 rhs=Xall[:, g * NP_:(g + 1) * NP_],
                                 start=True, stop=True)
            nc.scalar.activation(hs[layer][:, g4 * NP_: (g4 + gcnt) * NP_],
                                 agg[:, :gcnt * NP_], AF.Tanh, bias=bl)

    # ---- layer 3: per-node sort key (pre-tanh) ----
    vps = cx.phw.tile([128, 512], F32, tag="hw4")
    for i in range(bcnt):
        nc.tensor.matmul(vps[:NP_, i:i + 1], lhsT=h3[:, i * NP_:(i + 1) * NP_],
                         rhs=W["w3"], start=True, stop=True)
    vsb = cx.sm.tile([NP_, BG], F32, tag="vsb")
    nc.vector.tensor_copy(vsb[:, :bcnt], vps[:NP_, :bcnt])
    h4ps = cx.pag.tile([128, 512], F32, tag="agg")
    for i in range(bcnt):
        nc.tensor.matmul(h4ps[:NP_, i:i + 1], lhsT=Xall[:, i * NP_:(i + 1) * NP_],
                         rhs=vsb[:, i:i + 1], start=True, stop=True)
    h4blk = cx.sm.tile([NP_, BG], F32, tag="h4blk")
    nc.scalar.activation(h4blk[:, :bcnt], h4ps[:NP_, :bcnt], AF.Copy, bias=cx.b3)

    # ---- transpose keys to [graphs, nodes], top-40 via 5x top-8 ----
    h4Tps = cx.ptp.tile([128, 128], F32, tag="ptrans")
    nc.tensor.transpose(h4Tps[:bcnt, :NP_], h4blk[:, :bcnt], cx.ident[:NP_, :NP_])
    h4T = cx.sm.tile([BG, NP_], F32, tag="h4T")
    nc.vector.tensor_copy(h4T[:bcnt, :], h4Tps[:bcnt, :NP_])

    vals = cx.sm.tile([BG, K], F32, tag="vals")
    idxs32 = cx.sm.tile([BG, K], U32, tag="idxs32")
    idx16 = cx.sm.tile([BG, K], I16, tag="idx16")
    idxf = cx.sm.tile([BG, K], F32, tag="idxf")
    scr_idx = cx.dp.tile([BG, K], I16, tag="scr_idx")
    if bcnt < BG:
        nc.vector.memset(vals[:], 0.0)
        nc.vector.memset(idx16[:], 0)
    for r in range(5):
        nc.vector.max(vals[:bcnt, 8 * r:8 * r + 8], h4T[:bcnt, :])
        nc.vector.max_index(idxs32[:bcnt, 8 * r:8 * r + 8],
                            vals[:bcnt, 8 * r:8 * r + 8], h4T[:bcnt, :])
        if r < 4:
            nc.vector.match_replace(h4T[:bcnt, :], vals[:bcnt, 8 * r:8 * r + 8],
                                    h4T[:bcnt, :], -1e30)
        if last:
            # final block: convert + write this round's 8 index columns
            # immediately so the scratch-write latency hides under the rounds
            sl8 = slice(8 * r, 8 * r + 8)
            nc.vector.tensor_copy(idxf[:bcnt, sl8], idxs32[:bcnt, sl8])
            nc.vector.tensor_scalar_add(idxf[:bcnt, sl8], idxf[:bcnt, sl8],
                                        W["gbase"][:bcnt, :])
            nc.vector.tensor_copy(idx16[:bcnt, sl8], idxf[:bcnt, sl8])
            nc.sync.dma_start(out=scr_idx[:, sl8], in_=idx16[:, sl8])

    if not last:
        nc.vector.tensor_copy(idxf[:bcnt, :], idxs32[:bcnt, :])
        nc.vector.tensor_scalar_add(idxf[:bcnt, :], idxf[:bcnt, :],
                                    W["gbase"][:bcnt, :])
        nc.vector.tensor_copy(idx16[:bcnt, :], idxf[:bcnt, :])

    # ---- roundtrip through DRAM: wrapped idx layout + flat vals ----
    if not last:
        nc.sync.dma_start(out=scr_idx[:], in_=idx16[:])
    iw128 = cx.sm.tile([128, NH // 16], I16, tag="iw128")
    wrapped = scr_idx[:].rearrange("g k -> (g k)").rearrange("(c p) -> p c", p=16)
    for c in range(8):
        eng = nc.gpsimd if (last and c >= 5) else nc.sync
        eng.dma_start(out=iw128[16 * c:16 * (c + 1), :], in_=wrapped)

    nc.scalar.activation(vals[:bcnt, :], vals[:bcnt, :], AF.Tanh)
    scr_val = cx.dp.tile([BG, K], F32, tag="scr_val")
    nc.sync.dma_start(out=scr_val[:], in_=vals[:])
    h4row = cx.ld.tile([1, NH], F32R, tag="h4row")
    nc.sync.dma_start(out=h4row[:],
                      in_=scr_val[:].rearrange("g k -> (g k)").rearrange(
                          "(o n) -> o n", o=1).bitcast(F32R))
    s["iw128"], s["h4row"] = iw128, h4row


def _emit_gath(cx, b, bcnt, last=False):
    """Feature gathers for block b (Pool; consumed by conv1 a block later)."""
    nc = cx.nc
    s = cx.state[b]
    ncols = bcnt * NP_
    nki = ceil16(bcnt * K)
    grs = []
    for c in range(3):
        gat = cx.gta.tile([128, NH], F32, tag=f"gath{c}")
        nc.gpsimd.ap_gather(gat[:, :nki], s["hs"][c][:, :ncols],
                            s["iw128"][:, :nki // 16],
                            channels=128, num_elems=ncols, d=1, num_idxs=nki)
        gr = cx.gth.tile([128, NH], F32R, tag=f"gr{c}")
        # final block: DVE is idle at the drain, so the f32r casts pipeline
        # against the next gather instead of serializing on Pool
        ceng = nc.vector if last else nc.gpsimd
        ceng.tensor_copy(gr[:, :nki], gat[:, :nki])
        grs.append(gr)
    s["grs"] = grs


def _emit_conv1(cx, b, g0, bcnt):
    """conv1 for block b (gathers ran a block earlier -> PE never stalls)."""
    nc, W = cx.nc, cx.W
    s = cx.state[b]
    grs = s["grs"]
    nk = bcnt * K
    hof = g0 * K
    for ch0 in range(0, nk, 320):
        cw = min(320, nk - ch0)
        cps = cx.phw.tile([128, 512], F32, tag="hw4")
        for c in range(3):
            nc.tensor.matmul(cps[:16, :cw], lhsT=W["cw1c"][:, 16 * c:16 * (c + 1)],
                             rhs=grs[c][:, ch0:ch0 + cw], start=(c == 0), stop=False)
        nc.tensor.matmul(cps[:16, :cw], lhsT=W["cw1c3"],
                         rhs=s["h4row"][:1, ch0:ch0 + cw], start=False, stop=True)
        nc.scalar.activation(cx.Y1[:, hof + ch0: hof + ch0 + cw],
                             cps[:16, :cw], AF.Relu, bias=W["cb1"])


def _emit_tail_chunk(cx, ch, goff=0, gcnt=32):
    """Maxpool + conv2 + partial mlp1 + finish for graph slots
    [32*ch+goff, 32*ch+goff+gcnt) (per-graph independent -> splittable)."""
    nc, W = cx.nc, cx.W
    g0 = 32 * ch + goff
    Y2c = cx.st.tile([16, 640], F32R, tag="Y2c")
    y1v = cx.Y1[:, 40 * g0:40 * (g0 + gcnt)].rearrange(
        "p (m two) -> p m two", two=2)
    nc.vector.tensor_tensor(Y2c[:, :20 * gcnt], y1v[:, :, 0], y1v[:, :, 1],
                            op=ALU.max)

    # conv2: 5 shifted matmuls over the part's graph slots; pad tiny parts
    # to 16 slots so the fp32r matmul hits the >=256-free-dim fast path
    # (padded slots read stale-but-finite Y2c cols and are never consumed)
    gmm = max(gcnt, 16)
    cps = cx.pms.tile([32, 512], F32, tag="cps")
    for j in range(5):
        rhs = Y2c[:, :20 * gmm].rearrange("p (g t) -> p g t", t=20)[:, :, j:j + 16]
        nc.tensor.matmul(cps[:, :16 * gmm], lhsT=W[f"cw2j_{j}"], rhs=rhs,
                         start=(j == 0), stop=(j == 4))
    Y3c = cx.st.tile([32, 512], F32, tag="Y3c")
    nc.scalar.activation(Y3c[:, :16 * gcnt], cps[:, :16 * gcnt],
                         AF.Relu, bias=W["cb2"])

    # mlp1 partial: accumulate this part's columns of zps
    for t in range(16):
        rhs = Y3c[:, :16 * gcnt].rearrange("p (g t) -> p g t", t=16)[:, :, t]
        nc.tensor.matmul(cx.zps[:, g0:g0 + gcnt],
                         lhsT=W[f"mW1s_{t}"], rhs=rhs,
                         start=(t == 0), stop=(t == 15))

    # finish this part's graphs: relu -> mW2 -> +mb2 -> DMA out
    Z1 = cx.st.tile([128, 32], F32, tag="Z1")
    nc.scalar.activation(Z1[:, :gcnt], cx.zps[:, g0:g0 + gcnt],
                         AF.Relu, bias=W["mb1"])
    ypst = cx.pms.tile([32, 512], F32, tag="cps")
    nc.tensor.matmul(ypst[:1, :gcnt], lhsT=W["mW2"], rhs=Z1[:, :gcnt],
                     start=True, stop=True)
    ysb = cx.st.tile([1, 32], F32, tag="ysb")
    nc.scalar.activation(ysb[:, :gcnt], ypst[:1, :gcnt], AF.Copy, bias=cx.mb2)
    nc.sync.dma_start(out=cx.yout[:1, g0:g0 + gcnt], in_=ysb[:, :gcnt])


# ======================================================================
# public entry point: kernel(**inputs) -> np.ndarray [1000, 1]
# ======================================================================
NUM_CORES = 8
NUM_GRAPHS = 1000
G_PER_CORE = NUM_GRAPHS // NUM_CORES  # 125

_cache = {}


def _get_compiled(b3_val, mb2_val):
    key = "nc"
    if key not in _cache:
        from concourse import bacc
        nc = bacc.Bacc("TRN2", target_bir_lowering=False, debug=False,
                       num_devices=NUM_CORES, dynamic_dma_scratch_size=16384)
        build_kernel(nc, G_PER_CORE, b3_val, mb2_val)
        nc.compile()
        _cache[key] = nc
    return _cache[key]


def kernel(x, z, edge_index, batch, z_table, W0, b0, W1, b1, W2, b2, W3, b3,
           cw1, cb1, cw2, cb2, mW1, mb1, mW2, mb2, **_unused):
    x = np.asarray(x, np.float32)
    z = np.asarray(z)
    ei = np.asarray(edge_index)
    weights = dict(z_table=np.asarray(z_table), W0=np.asarray(W0),
                   b0=np.asarray(b0), W1=np.asarray(W1), b1=np.asarray(b1),
                   W2=np.asarray(W2), b2=np.asarray(b2), W3=np.asarray(W3),
                   b3=np.asarray(b3), cw1=np.asarray(cw1), cb1=np.asarray(cb1),
                   cw2=np.asarray(cw2), cb2=np.asarray(cb2), mW1=np.asarray(mW1),
                   mb1=np.asarray(mb1), mW2=np.asarray(mW2), mb2=np.asarray(mb2))

    # dense per-graph adjacency counts: row (g, d), col s  (edges are intra-graph)
    src = ei[0].astype(np.int64)
    dst = ei[1].astype(np.int64)
    code = dst * NP_ + (src % NP_)  # == (g*100 + d)*100 + s
    counts = np.bincount(code, minlength=NUM_GRAPHS * NP_ * NP_).astype(np.float32)
    counts = counts.reshape(NUM_GRAPHS, NP_, NP_)

    b3v = float(np.asarray(b3).reshape(-1)[0])
    mb2v = float(np.asarray(mb2).reshape(-1)[0])
    nc = _get_compiled(b3v, mb2v)

    in_maps = []
    for c in range(NUM_CORES):
        g0 = c * G_PER_CORE
        n0 = g0 * NP_
        n1 = n0 + G_PER_CORE * NP_
        m, _, _ = pack_core_inputs(x[n0:n1], z[n0:n1], counts[g0:g0 + G_PER_CORE],
                                   weights)
        in_maps.append(m)

    from concourse.bass_utils import run_bass_kernel_spmd
    res = run_bass_kernel_spmd(nc, in_maps, list(range(NUM_CORES)))
    y = np.concatenate([res.results[c]["y"][0, :G_PER_CORE]
                        for c in range(NUM_CORES)])
    return y.reshape(NUM_GRAPHS, 1).astype(np.float32)
